# revision 42
# baseline (speedup 1.0000x reference)
"""Multi-head attention (B=2, S=2048, D=1024, H=16) on 8 TRN2 NeuronCores.

Sharding: core c handles batch b = c//4 and heads [4*(c%4), 4*(c%4)+4) —
tensor-parallel over heads x data-parallel over batch.  Each core computes a
partial output projection (its heads' contribution); the host sums the 4
partials per batch and adds b_out.

On-device layout (per core; all matmul operands bf16, fp32 PSUM math):
  - qk projection computed transposed: qkT [512, S] bf16 with row chunks
    [q_h0|q_h1, k_h0|k_h1, q_h2|q_h3, k_h2|k_h3].  Phase 1 computes only the
    first head-pair's q,k (m=0,1), kc-outer across all 8 PSUM banks so
    matmuls start as each yT DMA chunk lands; q2/k2/q3/k3 and the v
    projection drain as PE filler inside the first attention blocks.
  - scores computed transposed: expT[sk, sq] = exp(0.125 * kT.T @ qT).
    Exps for 6 of 8 score chunk-pairs run on the ACT engine (table exp);
    the other 2 run on the DVE as a Schraudolph fast-exp (bf16 bits =
    score*log2e*128 + 16250.5 written through an int16 cast) — linear in
    the score so the ~+-12 score tails cannot overflow, and the ~3% PWL
    error largely cancels in the softmax ratio.  bf16 (not fp8) because
    exp reaches ~2e5 in the tails, far beyond fp8e4m3's 448 max.  The
    {1,5} ACT/DVE split is a measured local optimum: more DVE exps congest
    the DVE and delay the copies that release attn@v PSUM; all-ACT makes
    exp the per-slot pacer again.
  - attn@v: valuesT_unnorm [65, sq] = v_aug.T @ expT accumulated over sk
    chunks in reversed order (only the first matmul waits on the exps);
    the ones-column of v_aug makes PSUM row 64 the softmax denominator.
    Each chain is split in half and issued across 4 score-slots mid-block
    so the next block's exps never wait on an ex-buffer WAR hazard and the
    ACT engine never starves; the last block runs forward-order in-block.
  - normalization: per-(pair, sq-block) staged chain merging both heads.
    The denominator rows bounce through DRAM to be re-read as [128, 2, 4]
    so the DVE reciprocal runs on 128 lanes (a [1,512] reciprocal is 8
    cycles/elem on one lane = ~4.3us), bounced back as a 4KB row, and
    broadcast across 64 partitions by GPSIMD partition_broadcast (daisy
    chain, ~1.8us, SBUF-to-SBUF — a stride-0 DMA broadcast takes ~5us
    and a K=1 PE matmul broadcast ladders the in-order PE through PSUM
    WAR semaphores); DVE mul -> vT bf16.  The chain advances ONE stage
    per score slot (append mj3, rt / recip+d2 / d2r / pbcast / vtmp+vT)
    so every hop has >=1 slot to land before its consumer issues.  In
    the tail the first four stages are emitted BEFORE the outproj
    drains: the DVE is strict FIFO, so the chain's tiny reciprocal must
    sit ahead of the ost casts or it waits ~10us behind them.
  - output projection per sq block as soon as both pairs' vT slices land,
    deferred one extra block so its inputs' DMA chain is certain to have
    landed (the PE is strictly in-order; a not-ready filler blocks it);
    PSUM->SBUF copies split 3 ACT : 5 DVE; the two 512-col halves merge
    into one 256KB store.
  - DMA queues: everything posts from the Sync engine (hardware DGE
    queue, 16 engines) — gpsimd/vector dma_start land on a software
    queue (~60GB/s) that is only safe for tiny transfers.  The v
    projection is split by head pair: pair-1's v is first consumed by
    attn@v(block 4) during block 5, so its 16 groups defer to blocks
    3-4, which otherwise run out of PE filler (outproj only arrives
    from block 6) and idle ~1us/slot behind the ACT exps.
  - HAM warm-up: the PE clock defaults to 1.2GHz and only reaches 2.4GHz
    after ~3.4us of sustained matmul activity (and re-throttles after
    idle windows).  A handful of dummy matmuls on a memset tile run
    during the DMA-bound head so phase 1 starts warm; the phase1->block0
    transition splits its 8 bias copies between ACT and DVE and fills
    the PE with vproj groups so the HAM never sees an idle window there.
  - scheduling: the PE is the bottleneck engine (~185us busy).  Non-score
    PE work lives in a filler queue with per-block drain budgets
    (KDRB=12,4,4,8,8: pair-0 vprojs must all land in block 0 — block 1
    attn@v reads them and the in-order PE would deadlock-stall on
    fillers emitted behind it — m2/m3 projections in blocks 1-2,
    pair-1 vprojs in blocks 3-4).
  - measured ~217-222us (run-to-run HAM/DMA-phase noise ~3us); engine
    balance knobs: KDVEMJ/KDVEMJ1 (DVE exp slots; {1,5}/{1,5} best —
    more DVE exps add score->exp queue latency behind the DVE's chain
    work and stall the in-order PE even though raw ACT load drops),
    KOSTACT (ost copy ACT share), KWARM, KPREVP, KDRB.
"""
import sys

sys.path.insert(0, "/opt/trn_rl_repo")

import numpy as np

B, S, D = 2, 2048, 1024
H, Hd = 16, 64
P = 128
NKC = D // P      # 8 contraction chunks for the projections
NSC = S // P      # 16 sequence chunks of 128
SQB = 512         # sq block size
NSQB = S // SQB   # 4

_CACHE = {}


def _build_nc():
    import concourse.mybir as mybir
    import concourse.tile as tile
    from concourse import bacc

    f32 = mybir.dt.float32
    bf16 = mybir.dt.bfloat16
    fp8 = mybir.dt.float8e4
    AF = mybir.ActivationFunctionType
    DR = mybir.MatmulPerfMode.DoubleRow

    nc = bacc.Bacc(None, target_bir_lowering=False, debug=False)

    yT_d = nc.dram_tensor("yT", [D, S], bf16, kind="ExternalInput")[:]
    Wqk_d = nc.dram_tensor("WqkT", [D, 512], bf16, kind="ExternalInput")[:]
    bqk_d = nc.dram_tensor("bqk", [P, 4], f32, kind="ExternalInput")[:]
    Wv_d = nc.dram_tensor("WvT", [D, 256], bf16, kind="ExternalInput")[:]
    Wout_d = nc.dram_tensor("WoutT", [256, D], bf16, kind="ExternalInput")[:]
    out_d = nc.dram_tensor("out", [S, D], bf16, kind="ExternalOutput")[:]
    import os
    dbg = os.environ.get("KDBG") == "1"
    # Schraudolph fast-exp on the DVE for a subset of score chunks: bf16
    # bit pattern = y*2^7 + (127*2^7 - sigma), y = score*log2(e).  Linear in
    # the score (no overflow even at score ~ +-12); rel err ~ +-3% piecewise
    # linear, which the softmax normalization averages out.  Offloading 2 of
    # 8 chunks per block cuts the ACT exp floor by 25%.
    dve_mjs = {int(x) for x in
               os.environ.get("KDVEMJ", "1,5").split(",") if x != ""}
    dve_mjs_p1 = {int(x) for x in
                  os.environ.get("KDVEMJ1", "1,5").split(",") if x != ""}
    c1 = float(os.environ.get("KEXPC1", "16250.5"))
    n_warm = int(os.environ.get("KWARM", "6"))
    gp_norm = int(os.environ.get("KGPNORM", "1"))
    n_prevp = int(os.environ.get("KPREVP", "4"))
    ost_act = {int(x) for x in
               os.environ.get("KOSTACT", "0,3,6").split(",") if x != ""}
    if dbg:
        vT_dump = nc.dram_tensor("vT_dump", [P, 2, S], f32,
                                 kind="ExternalOutput")[:]
        den_dump = nc.dram_tensor("den_dump", [2, NSQB, 2, SQB], f32,
                                  kind="ExternalOutput")[:]
        rec_dump = nc.dram_tensor("rec_dump", [2, NSQB, 2, SQB], f32,
                                  kind="ExternalOutput")[:]

    with tile.TileContext(nc) as tc:
        with (
            tc.tile_pool(name="const", bufs=1) as const,
            tc.tile_pool(name="persist", bufs=1) as persist,
        ):
            qkT_sb = persist.tile([P, 4, S], bf16)
            # v_aug per sk-chunk/head: cols 0-63 = v, col 64 = 1.0 (the
            # ones column turns the attn@v matmul's row 64 into the softmax
            # denominator).  bf16: scores reach ~12 in the tails so exp
            # goes up to ~2e5 — far beyond fp8 range.
            v_sb = persist.tile([P, NSC, 4, 65], bf16)
            vT_sb = persist.tile([P, 2, S], bf16)
            ones_f32 = const.tile([P, 1], f32)
            nc.any.memset(ones_f32[:], 1.0)
            nc.vector.tensor_copy(
                v_sb[:, :, :, 64:65],
                ones_f32.unsqueeze(1).unsqueeze(1).to_broadcast(
                    (P, NSC, 4, 1)))

            # ---- warm-up: dummy matmuls on a memset tile keep the PE HAM
            # activity monitor busy during the DMA-bound head so phase 1
            # runs at 2.4GHz instead of the cold 1.2GHz default.  They sit
            # ahead of phase 1 in the in-order PE queue, so the count is
            # sized to end roughly when the first yT chunk lands. ----
            warm_w = const.tile([P, 512], bf16)
            nc.vector.memset(warm_w[:], 0.25)
            # 64-wide ones row: lhsT of the K=1 broadcast matmul that
            # replicates the reciprocal row across 64 partitions on the PE
            # (a stride-0 DMA broadcast of the same data takes ~2.5-5us;
            # the matmul takes ~430ns and lands in PSUM for the DVE mul)
            rb_ones = const.tile([P, 64], f32)
            nc.vector.memset(rb_ones[:], 1.0)
            if n_warm:
                warm_ps = tc.alloc_tile_pool(name="warmps", bufs=1,
                                             space="PSUM")
                psw = warm_ps.tile([P, 512], f32)
                for _ in range(n_warm):
                    nc.tensor.matmul(psw[:], warm_w[:, 0:128], warm_w[:],
                                     start=True, stop=True)
                warm_ps.release()

            # ---- DMA order matters for the head: yT/Wqk chunks first (the
            # phase-1 proj gates the first exp; yT kc=0 split in 4 so the
            # first matmul can start ~1us after DMA boot), Wv next (vproj
            # fillers fire right after phase 1), Wqk m2/m3 half after
            # (needed from block ~1), Wout last (needed from block 4). ----
            p1 = ctx_p1 = tc.alloc_tile_pool(name="p1", bufs=1)
            Wqk_sb = p1.tile([P, NKC, 512], bf16)
            yT_sb = p1.tile([P, NKC, S], bf16)
            yTr = yT_d.rearrange("(kc p) s -> p kc s", p=P)
            Wqkr = Wqk_d.rearrange("(kc p) e -> p kc e", p=P)
            bqk_sb = const.tile([P, 4], f32)
            nc.sync.dma_start(bqk_sb[:], bqk_d)
            nc.sync.dma_start(Wqk_sb[:, 0, 0:256], Wqkr[:, 0, 0:256])
            for sb in range(4):
                nc.sync.dma_start(yT_sb[:, 0, sb * 512:(sb + 1) * 512],
                                  yTr[:, 0, sb * 512:(sb + 1) * 512])
            for kc in range(1, NKC):
                nc.sync.dma_start(Wqk_sb[:, kc, 0:256], Wqkr[:, kc, 0:256])
                nc.sync.dma_start(yT_sb[:, kc, :], yTr[:, kc, :])
            Wv_sb = p1.tile([P, NKC, 256], bf16)
            nc.sync.dma_start(
                Wv_sb[:], Wv_d.rearrange("(kc p) e -> p kc e", p=P))
            nc.sync.dma_start(Wqk_sb[:, :, 256:512], Wqkr[:, :, 256:512])
            Wout_sb = const.tile([P, 2, D], bf16)
            nc.sync.dma_start(Wout_sb[:],
                              Wout_d.rearrange("(kc p) e -> p kc e", p=P))

            # ---- phase 1: q,k projection for head pair 0 (m=0,1),
            # kc-outer across 8 psum banks so matmuls start as each yT
            # chunk lands; at the last kc round each (m, sb) group's bias
            # copy is emitted immediately, k-then-q per sb, so the first
            # score block unblocks as early as possible. ----
            with tc.tile_pool(name="p1ps", bufs=8, space="PSUM") as p1ps:
                # bank order = downstream-consumer order: banks 0-1 are
                # recycled first (p2sh: the pre-block vproj groups), banks
                # 2-7 by the score psum pool slot by slot.  Allocating and
                # bias-draining the phase-1 groups in this same order
                # makes every WAR release sequential — previously the
                # first vproj sat ~1.5us behind late bias copies and the
                # PE idled into a HAM re-throttle at the transition.
                # p2sh recycles the LAST p1ps banks, so the two groups
                # it inherits — (k,sb0),(q,sb0), which are also what the
                # first scores need — allocate last but bias FIRST; the
                # score pool inherits banks 0-5 in slot order, matched by
                # the bias sequence below
                order_alloc = [(1, 1), (1, 2), (1, 3), (0, 1),
                               (0, 2), (0, 3), (1, 0), (0, 0)]
                order_bias = [(1, 0), (0, 0), (1, 1), (1, 2),
                              (0, 1), (1, 3), (0, 2), (0, 3)]
                tiles8 = {ms: p1ps.tile([P, 512], f32, tag="proj",
                                        name="ps01") for ms in order_alloc}
                ps_mm = [[tiles8[(m, sb)] for sb in range(4)]
                         for m in range(2)]
                for kc in range(NKC - 1):
                    for m in range(2):
                        for sb in range(4):
                            nc.tensor.matmul(
                                ps_mm[m][sb][:],
                                Wqk_sb[:, kc, m * P:(m + 1) * P],
                                yT_sb[:, kc, sb * 512:(sb + 1) * 512],
                                start=(kc == 0), stop=False)
                # all k (m=1) groups first, then q; the bias copies
                # alternate ACT/DVE so the transition into block 0 is not
                # serialized on one engine (the old all-DVE chain left the
                # PE idle ~4us and tripped the HAM re-throttle).
                for j, (m, sb) in enumerate(order_bias):
                    nc.tensor.matmul(
                        ps_mm[m][sb][:],
                        Wqk_sb[:, NKC - 1, m * P:(m + 1) * P],
                        yT_sb[:, NKC - 1, sb * 512:(sb + 1) * 512],
                        start=False, stop=True)
                    dst = qkT_sb[:, m, sb * 512:(sb + 1) * 512]
                    if j % 2 == 0:
                        nc.scalar.add(dst, ps_mm[m][sb][:],
                                      bqk_sb[:, m:m + 1])
                    else:
                        nc.vector.tensor_scalar_add(
                            dst, ps_mm[m][sb][:], bqk_sb[:, m:m + 1])

            # ---- phase 2: attention, ACT-paced; PE filler queue ----
            with (
                tc.tile_pool(name="p2e", bufs=4) as p2e,
                tc.tile_pool(name="p2s", bufs=2) as p2s,
                tc.tile_pool(name="p2ps", bufs=2, space="PSUM") as p2ps,
                tc.tile_pool(name="p2dram", bufs=8, space="DRAM") as p2dram,
                tc.tile_pool(name="p2sh", bufs=2, space="PSUM") as p2sh,
            ):
                filler = []
                filler_next = []

                def vproj_group(sc, pair):
                    # split by head pair: pair 1's v columns are first
                    # consumed by attn@v(block 4) which runs in block 5,
                    # so its 16 groups defer to blocks 3-4 as PE filler —
                    # without them those blocks idle ~1us/slot behind the
                    # ACT exps (no outproj arrives before block 6) and
                    # trip the HAM re-throttle
                    def run():
                        psv = p2sh.tile([P, 512], f32, tag="sh", name="psv")
                        for kc in range(NKC):
                            nc.tensor.matmul(
                                psv[:, 0:128],
                                yT_sb[:, kc, sc * P:(sc + 1) * P],
                                Wv_sb[:, kc, pair * 128:(pair + 1) * 128],
                                start=(kc == 0), stop=(kc == NKC - 1))
                        nc.vector.tensor_copy(
                            v_sb[:, sc, 2 * pair:2 * pair + 2, 0:64],
                            psv[:, 0:128].rearrange("p (i d) -> p i d",
                                                    i=2))
                    return run

                def proj_group(m, sb):
                    def run():
                        ps = p2sh.tile([P, 512], f32, tag="sh", name="psqk")
                        for kc in range(NKC):
                            nc.tensor.matmul(
                                ps[:],
                                Wqk_sb[:, kc, m * P:(m + 1) * P],
                                yT_sb[:, kc, sb * 512:(sb + 1) * 512],
                                start=(kc == 0), stop=(kc == NKC - 1))
                        # ACT: these fire in blocks 1-3 where the DVE is
                        # loaded with exps but the ACT has slack
                        nc.scalar.add(
                            qkT_sb[:, m, sb * 512:(sb + 1) * 512],
                            ps[:], bqk_sb[:, m:m + 1])
                    return run

                def outproj_group(sc):
                    def run():
                        ost = p2s.tile([P, 2, 512], bf16, tag="ost",
                                       name="ost", bufs=3)
                        for nb in range(2):
                            pso = p2sh.tile([P, 512], f32, tag="sh",
                                            name="pso")
                            for kc in range(2):
                                nc.tensor.matmul(
                                    pso[:],
                                    vT_sb[:, kc, sc * P:(sc + 1) * P],
                                    Wout_sb[:, kc,
                                            nb * 512:(nb + 1) * 512],
                                    start=(kc == 0), stop=(kc == 1))
                            # split the PSUM->SBUF copies 3 ACT : 5 DVE —
                            # the ACT carries 10 of 16 exps in the p=1
                            # blocks where these run
                            if (2 * sc + nb) % 8 in ost_act:
                                nc.scalar.copy(ost[:, nb, :], pso[:])
                            else:
                                nc.vector.tensor_copy(ost[:, nb, :],
                                                      pso[:])
                        # one merged 256KB store per row chunk on the Sync
                        # HARDWARE dma queue (gpsimd posts go to a slow
                        # software queue — only small descriptors go there)
                        nc.sync.dma_start(
                            out_d[sc * P:(sc + 1) * P, :]
                            .rearrange("p (a b) -> p a b", a=2), ost[:])
                    return run

                # per-block drain budgets pace the filler queue so the
                # m2/m3 projection groups land in blocks 2-3 (which would
                # otherwise have only ~10us of PE work against ~11us of
                # ACT exps and go ACT-bound, idling the PE and tripping
                # the HAM re-throttle); outproj fillers (from block 4 on)
                # are unbudgeted
                drain_budget = [99]
                budget_env = os.environ.get("KDRB", "12,4,4,8,8")
                drain_sched = [int(x) for x in budget_env.split(",")]

                def drain(k):
                    for _ in range(k):
                        if filler and drain_budget[0] > 0:
                            drain_budget[0] -= 1
                            filler.pop(0)()
                        elif filler_next:
                            filler_next.pop(0)()
                        else:
                            break

                # ---- normalize pipeline.  One chain per (pair, sq block),
                # merging both heads: denominator rows bounce through DRAM
                # to run the reciprocal on 128 lanes, then broadcast back.
                # The chain advances ONE stage per score slot so every DMA
                # has a full slot to land before its consumer issues, and
                # all descriptor posts go to the otherwise-idle GPSIMD
                # queue (the Sync engine serializes at ~600ns per post and
                # was the tail bottleneck). ----
                chains = []
                chain_by_key = {}

                def attn_v_tail(p, sqb, sub, psv2):
                    key = (p, sqb)
                    ch = chain_by_key.get(key)
                    if ch is None:
                        ch = {"p": p, "sqb": sqb, "stage": 0, "nsub": 0,
                              "vals": p2s.tile([64, 2, SQB], f32,
                                               tag="vals", name="vals",
                                               bufs=2),
                              "d1p": p2dram.tile([2, SQB], f32, name="d1p")}
                        chain_by_key[key] = ch
                    nc.vector.tensor_copy(ch["vals"][:, sub, :],
                                          psv2[0:64, :])
                    denrow = p2s.tile([P, SQB], f32, tag="den",
                                      name="den", bufs=4)
                    nc.vector.tensor_copy(denrow[64:65, :],
                                          psv2[64:65, :])
                    nc.sync.dma_start(ch["d1p"][sub:sub + 1, :],
                                      denrow[64:65, :])
                    if dbg:
                        nc.gpsimd.dma_start(
                            den_dump[p, sqb, sub].unsqueeze(0),
                            denrow[64:65, :])
                    ch["nsub"] += 1
                    if ch["nsub"] == 2:
                        chains.append(ch)

                def pop_chain():
                    if not chains:
                        return False
                    ch = chains[0]
                    st = ch["stage"]
                    p_, sqb_ = ch["p"], ch["sqb"]
                    if st == 0:
                        ch["rt"] = p2s.tile([P, 2, 4], f32, tag="rt",
                                            name="rt", bufs=2)
                        nc.sync.dma_start(
                            ch["rt"][:],
                            ch["d1p"].rearrange("s (p b) -> p s b", p=P))
                    elif st == 1:
                        rt2 = p2s.tile([P, 2, 4], f32, tag="rt2",
                                       name="rt2", bufs=2)
                        nc.vector.reciprocal(rt2[:], ch["rt"][:])
                        ch["d2p"] = p2dram.tile([2, SQB], f32, name="d2p")
                        nc.sync.dma_start(
                            ch["d2p"].rearrange("s (p b) -> p s b", p=P),
                            rt2[:])
                    elif st == 2:
                        # tiny (4KB) read of the reciprocal rows onto one
                        # partition; the PE broadcast matmul fans it out
                        ch["d2r"] = p2s.tile([1, 2, SQB], f32, tag="d2r",
                                             name="d2r", bufs=2)
                        nc.sync.dma_start(ch["d2r"][:], ch["d2p"]
                                          .unsqueeze(0))
                        if dbg:
                            for sub in range(2):
                                nc.sync.dma_start(
                                    rec_dump[p_, sqb_, sub].unsqueeze(0),
                                    ch["d2p"][sub:sub + 1, :])
                    elif st == 3:
                        # GPSIMD daisy-chain broadcast of the reciprocal
                        # rows from partition 0 to 64 partitions — SBUF to
                        # SBUF, no PSUM bank, no PE involvement, and the
                        # GPSIMD is otherwise idle (~1-2us vs ~5us for the
                        # stride-0 DMA broadcast, which was the tail's
                        # long pole)
                        ch["rbs"] = p2s.tile([64, 2, SQB], f32,
                                             tag="rbs", name="rbs",
                                             bufs=2)
                        nc.gpsimd.partition_broadcast(ch["rbs"][:],
                                                      ch["d2r"][:])
                    else:
                        sq = slice(sqb_ * SQB, (sqb_ + 1) * SQB)
                        # sub0's rows live on partitions 0-63 — the same
                        # partitions as its vT destination — so the DVE
                        # writes vT_sb directly; only sub1 needs the
                        # partition-shifting SBUF DMA (halves the vT
                        # round-trip latency that gates the outproj)
                        nc.vector.tensor_mul(vT_sb[0:64, p_, sq],
                                             ch["vals"][:, 0, :],
                                             ch["rbs"][:, 0, :])
                        vtmp = p2s.tile([64, SQB], bf16, tag="vtmp",
                                        name="vtmp", bufs=2)
                        nc.vector.tensor_mul(vtmp[:], ch["vals"][:, 1, :],
                                             ch["rbs"][:, 1, :])
                        nc.sync.dma_start(vT_sb[64:128, p_, sq], vtmp[:])
                        chains.pop(0)
                        del chain_by_key[(p_, sqb_)]
                        if p_ == 1:
                            # outproj lands in filler_next: drained next
                            # block, by which time the vT DMAs are certain
                            # to have completed (the PE is strictly
                            # in-order; a not-ready filler blocks it)
                            filler_next.extend(
                                outproj_group(sc)
                                for sc in range(sqb_ * 4, sqb_ * 4 + 4))
                        return True
                    ch["stage"] += 1
                    return True

                cur_psv2 = {}

                def attn_v_part(p, sqb, ex, sub, part, fwd=False):
                    """Half of one head's attn@v chain.  Normally reversed
                    (part 0 = upper sk chunks w/ group start) so only the
                    first matmul waits on ACT; the last block runs forward
                    (fwd=True) so it can overlap its own exps in-block.
                    Split in half so each PE burst fits inside the
                    score-tile lookahead and the ACT engine never starves.
                    Part 1 finishes the group and launches the
                    transposed-reciprocal normalize chain."""
                    i = 2 * p + sub
                    half = NSC // 2
                    if part == 0:
                        psv2 = p2sh.tile([P, SQB], f32, tag="sh",
                                         name="psv2")
                        cur_psv2[sub] = psv2
                        mks = range(0, half) if fwd else \
                            range(NSC - 1, half - 1, -1)
                        first = 0 if fwd else NSC - 1
                        for mk in mks:
                            nc.tensor.matmul(
                                psv2[0:65, :],
                                v_sb[:, mk, i, :],
                                ex[sub][:, mk, :],
                                start=(mk == first), stop=False)
                        return
                    psv2 = cur_psv2[sub]
                    mks = range(half, NSC) if fwd else \
                        range(half - 1, -1, -1)
                    lastmk = NSC - 1 if fwd else 0
                    for mk in mks:
                        nc.tensor.matmul(
                            psv2[0:65, :],
                            v_sb[:, mk, i, :],
                            ex[sub][:, mk, :],
                            start=False, stop=(mk == lastmk))
                    attn_v_tail(p, sqb, sub, psv2)

                # a few vproj groups run inline between phase 1 and block
                # 0: they only need yT+Wv (both landed) and keep the PE
                # busy while the ACT/DVE bias copies finish, so the HAM
                # never sees an idle window at the transition
                for sc in range(NSC - 1, NSC - 1 - n_prevp, -1):
                    vproj_group(sc, 0)()
                # priority-ordered by need time: pair-0 v chunks gate
                # attn@v(b0) at block-1 (all 16 must land by block 0's
                # end); m2/m3 gate block 4's scores (land by block 2);
                # pair-1 v chunks gate attn@v(b4) at block 5 (land by
                # block 4's end, filling the otherwise-bare blocks 3-4)
                filler.extend(vproj_group(sc, 0)
                              for sc in range(NSC - 1 - n_prevp, -1, -1))
                filler.extend(proj_group(m, sb)
                              for m in (2, 3) for sb in range(4))
                filler.extend(vproj_group(sc, 1)
                              for sc in range(NSC - 1, -1, -1))

                prev = None
                for p in range(2):
                    # p=1 blocks have less PE filler, so the ACT would pace
                    # them at 12 exps/block; shift more exps to the DVE
                    dmjs = dve_mjs if p == 0 else dve_mjs_p1
                    for sqb in range(NSQB):
                        bi = p * NSQB + sqb
                        drain_budget[0] = (drain_sched[bi]
                                           if bi < len(drain_sched) else 99)
                        last = (p == 1 and sqb == NSQB - 1)
                        sq = slice(sqb * SQB, (sqb + 1) * SQB)
                        exa = p2e.tile([P, NSC, SQB], bf16, tag="exp")
                        exb = p2e.tile([P, NSC, SQB], bf16, tag="exp")
                        ex = (exa, exb)
                        for mj in range(NSC // 2):
                            pss = [
                                p2ps.tile([P, 2, SQB], f32, tag="score",
                                          bufs=3, name="pss")
                                for _ in range(2)]
                            for half in range(2):
                                mk = 2 * mj + half
                                for sub in range(2):
                                    prt = slice(sub * 64, (sub + 1) * 64)
                                    nc.tensor.matmul(
                                        pss[sub][:, half, :],
                                        qkT_sb[prt, 2 * p + 1,
                                               mk * P:(mk + 1) * P],
                                        qkT_sb[prt, 2 * p, sq])
                            for sub in range(2):
                                if mj in dmjs:
                                    nc.vector.tensor_scalar(
                                        ex[sub][:, 2 * mj:2 * mj + 2, :]
                                        .bitcast(mybir.dt.int16),
                                        pss[sub][:],
                                        0.125 * 1.4426950408889634 * 128,
                                        c1,
                                        mybir.AluOpType.mult,
                                        mybir.AluOpType.add)
                                else:
                                    nc.scalar.activation(
                                        ex[sub][:, 2 * mj:2 * mj + 2, :],
                                        pss[sub][:], AF.Exp, scale=0.125)
                            # the normalize chain advances one stage per
                            # slot: append at mj3, then rt / recip+d2 /
                            # rbs / spacing / vtmp+vT across the next five
                            # slots (wrapping into the next block's mj0)
                            pop_chain()
                            # prev block's attn@v in four mid-block bursts
                            # (never at the boundary: next block's exps must
                            # not wait on an ex-buffer WAR hazard)
                            if prev is not None and mj < 4:
                                attn_v_part(*prev, sub=mj // 2, part=mj % 2)
                            elif last and mj == 7:
                                # sub0's upper half can start as soon as
                                # this block's own mj4-7 exps are issued
                                attn_v_part(p, sqb, ex, sub=0, part=1,
                                            fwd=True)
                            elif last and mj in (4, 5):
                                # last block: the lower-half attn@v chains
                                # run in-block forward order (their exps,
                                # mj0-3, are already issued; upper halves
                                # must wait for mj6/7's exps and run after
                                # the loop)
                                attn_v_part(p, sqb, ex, sub=mj - 4,
                                            part=0, fwd=True)
                            else:
                                drain(2)
                        prev = (p, sqb, ex)
                        filler.extend(filler_next)
                        del filler_next[:]
                attn_v_part(1, NSQB - 1, prev[2], sub=1, part=1, fwd=True)
                # the last chain's first stages go into the engine queues
                # BEFORE the sqb2 outproj drains: the DVE is strict FIFO,
                # so the chain's tiny reciprocal must sit ahead of the
                # outproj ost casts or it waits ~10us behind them; the
                # d2/d2r/broadcast stages run on Sync/GPSIMD and flow in
                # parallel with the outproj stream
                for _ in range(4):
                    pop_chain()
                # sqb2's outproj groups are ready (their vT landed during
                # block 7) — run them while the chain's DRAM bounces and
                # GPSIMD broadcast land
                drain(len(filler))
                while chains:
                    pop_chain()
                # bridge the remaining chain latency with dummy matmuls so
                # the HAM keeps the PE at 2.4GHz for the final outproj
                # burst (otherwise the ~5us idle re-throttles to 1.2GHz
                # and the tail runs at half speed)
                n_twarm = int(os.environ.get("KTWARM", "0"))
                if n_twarm:
                    tail_ps = p2ps.tile([P, 2, SQB], f32, tag="score",
                                        bufs=3, name="pss")
                    for _ in range(n_twarm):
                        nc.tensor.matmul(tail_ps[:, 0, :],
                                         warm_w[:, 0:128], warm_w[:],
                                         start=True, stop=True)
                while filler or filler_next:
                    drain(4)
                if dbg:
                    vT32 = p2s.tile([P, 2, S], f32, tag="vT32", name="vT32",
                                    bufs=1)
                    nc.vector.tensor_copy(vT32[:], vT_sb[:])
                    nc.sync.dma_start(vT_dump, vT32[:])

            ctx_p1.release()

    nc.compile()
    return nc


def _get_nc():
    if "nc" not in _CACHE:
        _CACHE["nc"] = _build_nc()
    return _CACHE["nc"]


def _host_prep(y, W_qkv, b_qkv, W_out, c):
    b = c // 4
    q = c % 4
    hs = [4 * q + i for i in range(4)]

    def Wrow(h, part):
        return W_qkv[h * 192 + part * 64: h * 192 + (part + 1) * 64]

    def brow(h, part):
        return b_qkv[h * 192 + part * 64: h * 192 + (part + 1) * 64]

    qk_rows = np.concatenate([
        Wrow(hs[0], 0), Wrow(hs[1], 0), Wrow(hs[0], 1), Wrow(hs[1], 1),
        Wrow(hs[2], 0), Wrow(hs[3], 0), Wrow(hs[2], 1), Wrow(hs[3], 1)],
        axis=0)
    bqk_flat = np.concatenate([
        brow(hs[0], 0), brow(hs[1], 0), brow(hs[0], 1), brow(hs[1], 1),
        brow(hs[2], 0), brow(hs[3], 0), brow(hs[2], 1), brow(hs[3], 1)],
        axis=0)
    import ml_dtypes

    bf = ml_dtypes.bfloat16
    WqkT = np.ascontiguousarray(qk_rows.T.astype(bf))        # [1024, 512]
    bqk = np.ascontiguousarray(bqk_flat.reshape(4, P).T)     # [128, 4]
    WvT = np.ascontiguousarray(
        np.concatenate([Wrow(h, 2) for h in hs], axis=0).T.astype(bf))
    dsl = np.concatenate([np.arange(h * 64, (h + 1) * 64) for h in hs])
    WoutT = np.ascontiguousarray(W_out[:, dsl].T.astype(bf))  # [256, 1024]
    yT = np.ascontiguousarray(y[b].T.astype(bf))             # [1024, 2048]
    return {"yT": yT, "WqkT": WqkT, "bqk": bqk, "WvT": WvT,
            "WoutT": WoutT}


def _gather(results, b_qkv, W_out, b_out):
    parts = [np.asarray(results[c]["out"], dtype=np.float32)
             for c in range(8)]
    # v-bias commutes through the output projection: fold it host-side
    bv_full = b_qkv.reshape(16, 3, 64)[:, 2, :].reshape(1024)
    bias = b_out + bv_full @ W_out.T
    return np.stack([
        parts[0] + parts[1] + parts[2] + parts[3] + bias,
        parts[4] + parts[5] + parts[6] + parts[7] + bias,
    ]).astype(np.float32)


def kernel(y, W_qkv, b_qkv, W_out, b_out):
    from concourse.bass_utils import run_bass_kernel_spmd

    y = np.ascontiguousarray(np.asarray(y, dtype=np.float32))
    W_qkv = np.ascontiguousarray(np.asarray(W_qkv, dtype=np.float32))
    b_qkv = np.ascontiguousarray(np.asarray(b_qkv, dtype=np.float32))
    W_out = np.ascontiguousarray(np.asarray(W_out, dtype=np.float32))
    b_out = np.asarray(b_out, dtype=np.float32)

    nc = _get_nc()
    in_maps = [_host_prep(y, W_qkv, b_qkv, W_out, c) for c in range(8)]
    res = run_bass_kernel_spmd(nc, in_maps, core_ids=list(range(8)))
    return _gather(res.results, b_qkv, W_out, b_out)

